# revision 9
# baseline (speedup 1.0000x reference)
"""2-layer GAT on 8 trn2 NeuronCores.

Strategy (self-contained, hardcoded for N=100000, E=3200000, 128->64->32):
 - Host: degree-sort nodes (desc), global blocks of 128, dealt round-robin to
   8 cores. Per-block edge budget K_j = max degree in global block j (shared
   across cores). Per-edge gather index tables built on host.
 - Program A: each core computes x@W1 (+attention dots) for its 1/8 of the
   nodes -> host concatenates the full feature table (node-id order).
 - Program B: per dst-block, K indirect DMAs (one offset per partition --
   the only HW-supported form) gather the edge rows, segment softmax +
   weighted mean via an edge-major multiply + binary-tree folds (a
   constant-1 slot in each table row makes the softmax denominator fall
   out of the same fold chain), project to layer-2 rows -> host
   interleaves the position-ordered table.
 - Program C: same aggregation for layer 2 + final row softmax.
"""

import sys
from contextlib import ExitStack

import numpy as np

sys.path.insert(0, "/opt/trn_rl_repo")

import ml_dtypes  # noqa: E402

import concourse.bass as bass  # noqa: E402
import concourse.bacc as bacc  # noqa: E402
import concourse.tile as tile  # noqa: E402
from concourse import mybir  # noqa: E402
from concourse.bass_utils import run_bass_kernel_spmd  # noqa: E402
from concourse.masks import make_identity  # noqa: E402

N = 100000
E = 3200000
IN_F, HID_F, OUT_F = 128, 64, 32
NEG = 0.2
CORES = 8
P = 128
NBLK = 98            # per-core dst blocks
NPC = NBLK * P       # 12544 per-core node slots
NSH = N // CORES     # 12500 nodes per core in program A
GSLOTS = NBLK * CORES * P  # 100352 global position slots

# layer-1 table row: [h x64 | one | pad | gs f32 | gd f32] = 70 bf16 elems
E1 = 70
T1 = E1 // 2         # 35 f32 words
GS1 = 33             # f32 word index of gs
GD1 = 34
MW1 = HID_F + 1      # 65: h + one (multiplied by attention weight)
SENT1 = N            # sentinel row in h1 table

# layer-2 table row: [h x32 | one | pad | gs f32 | gd f32] = 38 bf16 elems
E2 = 38
T2 = E2 // 2         # 19 f32 words
GS2 = 17
GD2 = 18
MW2 = OUT_F + 1      # 33
SENT2 = GSLOTS       # sentinel row in h2 table

bf = mybir.dt.bfloat16
f32 = mybir.dt.float32
i32 = mybir.dt.int32
AF = mybir.ActivationFunctionType
OP = mybir.AluOpType

LAST_RESULT = None
_CACHE = {}


# ----------------------------------------------------------------- host prep
def _host_prep(edge_index):
    src = np.asarray(edge_index[0], dtype=np.int64)
    dst = np.asarray(edge_index[1], dtype=np.int64)
    deg = np.bincount(dst, minlength=N).astype(np.int64) + 1  # incl self loop
    order = np.argsort(-deg, kind="stable")                   # global pos -> node
    degs = deg[order]
    Ks = [int(degs[j * CORES * P]) for j in range(NBLK)]
    Kmax = max(Ks)

    # edges grouped by dst
    eorder = np.argsort(dst, kind="stable")
    ssorted = src[eorder]
    dsorted = dst[eorder]
    counts = np.bincount(dst, minlength=N)
    starts = np.zeros(N, dtype=np.int64)
    starts[1:] = np.cumsum(counts)[:-1]

    pos_of_node = np.empty(N, dtype=np.int64)                 # node -> global pos
    pos_of_node[order] = np.arange(N)

    M = np.full((GSLOTS, Kmax), SENT1, dtype=np.int32)
    M[:N, 0] = order.astype(np.int32)                          # self loop at k=0
    slot_k = (np.arange(E) - starts[dsorted] + 1).astype(np.int64)
    M[pos_of_node[dsorted], slot_k] = ssorted.astype(np.int32)

    # layer-2 index: node id -> global position in the interleaved h2 table
    posfull = np.full(N + 1, SENT2, dtype=np.int32)
    posfull[:N] = pos_of_node.astype(np.int32)
    M2 = posfull[M]

    TOT = P * sum(Ks)
    idx1 = np.empty((CORES, TOT), dtype=np.int32)
    idx2 = np.empty((CORES, TOT), dtype=np.int32)
    for c in range(CORES):
        off = 0
        for j in range(NBLK):
            g0 = (j * CORES + c) * P
            K = Ks[j]
            idx1[c, off:off + P * K] = M[g0:g0 + P, :K].reshape(-1)
            idx2[c, off:off + P * K] = M2[g0:g0 + P, :K].reshape(-1)
            off += P * K
    return Ks, order, idx1, idx2


# ------------------------------------------------------------- device helpers
def _fold_sum(nc, tmp, k, mw):
    """Binary-tree fold of k groups of mw bf16 elems down to group 0."""
    while k > 1:
        if k % 2 == 1:
            nc.vector.tensor_tensor(
                out=tmp[:, 0:mw], in0=tmp[:, 0:mw],
                in1=tmp[:, (k - 1) * mw:k * mw], op=OP.add)
            k -= 1
        else:
            h = k // 2
            nc.vector.tensor_tensor(
                out=tmp[:, 0:h * mw], in0=tmp[:, 0:h * mw],
                in1=tmp[:, h * mw:k * mw], op=OP.add)
            k = h


def _mk_brep(nc, consts, psum, ones1, bsrc, fw, tag):
    brep = consts.tile([P, fw], f32)
    pb = psum.tile([P, fw], f32, tag=tag)
    nc.tensor.matmul(out=pb[:], lhsT=ones1[:], rhs=bsrc[:], start=True,
                     stop=True)
    nc.vector.tensor_copy(out=brep[:], in_=pb[:])
    return brep


def _agg_blocks(nc, sb, psum, Ks, KM, idx_dram, table_ap, ew, tw, mw, gsw, gdw,
                brep, wnext, ident, c2, h2o, outp):
    """Per-block aggregation. wnext!=None: layer 1 (project+emit h2 rows);
    else: layer 2 (final row softmax -> outp)."""
    fw = mw - 1
    off = 0
    for j in range(NBLK):
        K = Ks[j]
        idxt = sb.tile([P, K], i32, tag="idx", padded_shape=[P, KM])
        nc.sync.dma_start(
            out=idxt[:],
            in_=idx_dram.ap()[off:off + P * K].rearrange("(p k) -> p k", p=P))
        hg = sb.tile([P, K * ew], bf, tag="hg", padded_shape=[P, KM * ew])
        for k in range(K):
            nc.gpsimd.indirect_dma_start(
                out=hg[:, k * ew:(k + 1) * ew], out_offset=None,
                in_=table_ap,
                in_offset=bass.IndirectOffsetOnAxis(ap=idxt[:, k:k + 1],
                                                    axis=0))
        hgf = hg[:].bitcast(f32)
        # per-edge logits: z = gs[src] + gd[dst]; leaky relu; exp
        z = sb.tile([P, K], f32, tag="z", padded_shape=[P, KM])
        nc.vector.tensor_scalar(
            out=z[:].rearrange("p (k o) -> p k o", o=1),
            in0=hgf.rearrange("p (k t) -> p k t", t=tw)[:, :, gsw:gsw + 1],
            scalar1=hgf[:, gdw:gdw + 1], scalar2=None, op0=OP.add)
        zl = sb.tile([P, K], f32, tag="zl", padded_shape=[P, KM])
        nc.vector.scalar_tensor_tensor(out=zl[:], in0=z[:], scalar=NEG,
                                       in1=z[:], op0=OP.mult, op1=OP.max)
        ez = sb.tile([P, K], bf, tag="ez", padded_shape=[P, KM])
        nc.scalar.activation(out=ez[:], in_=zl[:], func=AF.Exp)
        # weighted rows (incl the ones slot -> denominator), tree-fold sum
        tmp = sb.tile([P, K * mw], bf, tag="tmp", padded_shape=[P, KM * mw])
        hg3 = hg[:].rearrange("p (k e) -> p k e", e=ew)[:, :, 0:mw]
        ez3 = ez[:].rearrange("p (k o) -> p k o", o=1).to_broadcast([P, K, mw])
        nc.vector.tensor_tensor(
            out=tmp[:].rearrange("p (k e) -> p k e", e=mw),
            in0=hg3, in1=ez3, op=OP.mult)
        _fold_sum(nc, tmp, K, mw)
        r = sb.tile([P, 1], f32, tag="r")
        nc.vector.reciprocal(out=r[:], in_=tmp[:, fw:fw + 1])
        o1 = sb.tile([P, fw], f32, tag="o1")
        nc.vector.scalar_tensor_tensor(out=o1[:], in0=tmp[:, 0:fw],
                                       scalar=r[:], in1=brep[:],
                                       op0=OP.mult, op1=OP.add)
        if wnext is not None:
            o1b = sb.tile([P, fw], bf, tag="o1b")
            nc.scalar.activation(out=o1b[:], in_=o1[:], func=AF.Relu)
            pt = psum.tile([fw, P], bf, tag="pt")
            nc.tensor.transpose(out=pt[:], in_=o1b[:], identity=ident[:])
            o1T = sb.tile([fw, P], bf, tag="o1T")
            nc.scalar.activation(out=o1T[:], in_=pt[:], func=AF.Copy)
            p34 = psum.tile([P, OUT_F + 2], f32, tag="p34")
            nc.tensor.matmul(out=p34[:], lhsT=o1T[:], rhs=wnext[:],
                             start=True, stop=True)
            th2 = sb.tile([P, E2], bf, tag="th2")
            nc.scalar.activation(out=th2[:, 0:OUT_F], in_=p34[:, 0:OUT_F],
                                 func=AF.Copy)
            nc.scalar.activation(out=th2[:, OUT_F:OUT_F + 2], in_=c2[:],
                                 func=AF.Copy)
            nc.vector.tensor_copy(
                out=th2[:, 2 * GS2:2 * GS2 + 4].bitcast(f32),
                in_=p34[:, OUT_F:OUT_F + 2])
            nc.sync.dma_start(out=h2o.ap()[j * P:(j + 1) * P, :], in_=th2[:])
        else:
            negm = sb.tile([P, 1], f32, tag="negm")
            nc.vector.tensor_reduce(out=negm[:], in_=o1[:],
                                    axis=mybir.AxisListType.X,
                                    op=OP.max, negate=True)
            e2 = sb.tile([P, fw], f32, tag="e2")
            nc.scalar.activation(out=e2[:], in_=o1[:], func=AF.Exp,
                                 bias=negm[:])
            ssum = sb.tile([P, 1], f32, tag="ssum")
            nc.vector.tensor_reduce(out=ssum[:], in_=e2[:],
                                    axis=mybir.AxisListType.X, op=OP.add)
            rs = sb.tile([P, 1], f32, tag="rs")
            nc.vector.reciprocal(out=rs[:], in_=ssum[:])
            of = sb.tile([P, fw], f32, tag="of")
            nc.vector.tensor_scalar(out=of[:], in0=e2[:], scalar1=rs[:],
                                    scalar2=None, op0=OP.mult)
            nc.sync.dma_start(out=outp.ap()[j * P:(j + 1) * P, :], in_=of[:])
        off += P * K


# --------------------------------------------------------------- program A
def _build_progA():
    nc = bacc.Bacc("TRN2", target_bir_lowering=False, debug=False,
                   enable_asserts=False, num_devices=CORES)
    xT = nc.dram_tensor("xt", [IN_F, NSH], bf, kind="ExternalInput")
    w1e = nc.dram_tensor("w1e", [IN_F, HID_F + 2], bf, kind="ExternalInput")
    h1o = nc.dram_tensor("h1o", [NSH, E1], bf, kind="ExternalOutput")

    with ExitStack() as ctx:
        tc = ctx.enter_context(tile.TileContext(nc))
        consts = ctx.enter_context(tc.tile_pool(name="consts", bufs=1))
        psum = ctx.enter_context(tc.tile_pool(name="psum", bufs=2,
                                              space="PSUM"))
        sb = ctx.enter_context(tc.tile_pool(name="sb", bufs=3))
        w1sb = consts.tile([IN_F, HID_F + 2], bf)
        nc.sync.dma_start(out=w1sb[:], in_=w1e.ap())
        c2 = consts.tile([P, 2], bf)
        nc.gpsimd.memset(c2[:, 0:1], 1.0)
        nc.gpsimd.memset(c2[:, 1:2], 0.0)

        CH = 2048
        for c0 in range(0, NSH, CH):
            nn = min(CH, NSH - c0)
            xt_t = sb.tile([IN_F, nn], bf, tag="xt", padded_shape=[IN_F, CH])
            nc.sync.dma_start(out=xt_t[:], in_=xT.ap()[:, c0:c0 + nn])
            for q0 in range(0, nn, P):
                qa = min(P, nn - q0)
                p66 = psum.tile([P, HID_F + 2], f32, tag="p66")
                nc.tensor.matmul(out=p66[:qa, :], lhsT=xt_t[:, q0:q0 + qa],
                                 rhs=w1sb[:], start=True, stop=True)
                th = sb.tile([P, E1], bf, tag="th")
                nc.scalar.activation(out=th[:qa, 0:HID_F],
                                     in_=p66[:qa, 0:HID_F], func=AF.Copy)
                nc.scalar.activation(out=th[:qa, HID_F:HID_F + 2],
                                     in_=c2[:qa, :], func=AF.Copy)
                nc.vector.tensor_copy(
                    out=th[:qa, 2 * GS1:2 * GS1 + 4].bitcast(f32),
                    in_=p66[:qa, HID_F:HID_F + 2])
                nc.sync.dma_start(out=h1o.ap()[c0 + q0:c0 + q0 + qa, :],
                                  in_=th[:qa, :])
    nc.compile()
    return nc


# --------------------------------------------------------------- program B
def _build_progB(Ks):
    TOT = P * sum(Ks)
    KM = max(Ks)
    nc = bacc.Bacc("TRN2", target_bir_lowering=False, debug=False,
                   enable_asserts=False, num_devices=CORES)
    h1t = nc.dram_tensor("h1t", [N + 1, E1], bf, kind="ExternalInput")
    w2e = nc.dram_tensor("w2e", [HID_F, OUT_F + 2], bf, kind="ExternalInput")
    b1d = nc.dram_tensor("b1d", [1, HID_F], bf, kind="ExternalInput")
    ix1 = nc.dram_tensor("ix1", [TOT], i32, kind="ExternalInput")
    h2o = nc.dram_tensor("h2o", [NPC, E2], bf, kind="ExternalOutput")

    with ExitStack() as ctx:
        tc = ctx.enter_context(tile.TileContext(nc))
        consts = ctx.enter_context(tc.tile_pool(name="consts", bufs=1))
        psum = ctx.enter_context(tc.tile_pool(name="psum", bufs=2,
                                              space="PSUM"))
        sb = ctx.enter_context(tc.tile_pool(name="sb", bufs=3))
        ident = consts.tile([P, P], bf)
        make_identity(nc, ident[:])
        ones1 = consts.tile([1, P], bf)
        nc.gpsimd.memset(ones1[:], 1.0)
        c2 = consts.tile([P, 2], bf)
        nc.gpsimd.memset(c2[:, 0:1], 1.0)
        nc.gpsimd.memset(c2[:, 1:2], 0.0)
        w2sb = consts.tile([HID_F, OUT_F + 2], bf)
        nc.sync.dma_start(out=w2sb[:], in_=w2e.ap())
        b1r = consts.tile([1, HID_F], bf)
        nc.sync.dma_start(out=b1r[:], in_=b1d.ap())
        b1rep = _mk_brep(nc, consts, psum, ones1, b1r, HID_F, "pb1")

        _agg_blocks(nc, sb, psum, Ks, KM, ix1, h1t.ap(), E1, T1, MW1, GS1,
                    GD1, b1rep, w2sb, ident, c2, h2o, None)
    nc.compile()
    return nc


# --------------------------------------------------------------- program C
def _build_progC(Ks):
    TOT = P * sum(Ks)
    KM = max(Ks)
    nc = bacc.Bacc("TRN2", target_bir_lowering=False, debug=False,
                   enable_asserts=False, num_devices=CORES)
    h2t = nc.dram_tensor("h2t", [GSLOTS + 1, E2], bf, kind="ExternalInput")
    b2d = nc.dram_tensor("b2d", [1, OUT_F], bf, kind="ExternalInput")
    ix2 = nc.dram_tensor("ix2", [TOT], i32, kind="ExternalInput")
    outp = nc.dram_tensor("outp", [NPC, OUT_F], f32, kind="ExternalOutput")

    with ExitStack() as ctx:
        tc = ctx.enter_context(tile.TileContext(nc))
        consts = ctx.enter_context(tc.tile_pool(name="consts", bufs=1))
        psum = ctx.enter_context(tc.tile_pool(name="psum", bufs=2,
                                              space="PSUM"))
        sb = ctx.enter_context(tc.tile_pool(name="sb", bufs=3))
        ones1 = consts.tile([1, P], bf)
        nc.gpsimd.memset(ones1[:], 1.0)
        b2r = consts.tile([1, OUT_F], bf)
        nc.sync.dma_start(out=b2r[:], in_=b2d.ap())
        b2rep = _mk_brep(nc, consts, psum, ones1, b2r, OUT_F, "pb2")

        _agg_blocks(nc, sb, psum, Ks, KM, ix2, h2t.ap(), E2, T2, MW2, GS2,
                    GD2, b2rep, None, None, None, None, outp)
    nc.compile()
    return nc


# ------------------------------------------------------------------- kernel
def kernel(x, edge_index, W1, att_src1, att_dst1, b1, W2, att_src2, att_dst2,
           b2, _trace=False):
    global LAST_RESULT
    x = np.asarray(x, dtype=np.float32)
    W1 = np.asarray(W1, dtype=np.float32)
    W2 = np.asarray(W2, dtype=np.float32)

    Ks, order, idx1, idx2 = _host_prep(np.asarray(edge_index))

    key = tuple(Ks)
    if key not in _CACHE:
        _CACHE[key] = (_build_progA(), _build_progB(Ks), _build_progC(Ks))
    ncA, ncB, ncC = _CACHE[key]

    bfnp = ml_dtypes.bfloat16
    xT = np.ascontiguousarray(x.T).astype(bfnp)
    w1ext = np.concatenate(
        [W1, (W1 @ np.asarray(att_src1, np.float32))[:, None],
         (W1 @ np.asarray(att_dst1, np.float32))[:, None]], axis=1).astype(bfnp)
    w2ext = np.concatenate(
        [W2, (W2 @ np.asarray(att_src2, np.float32))[:, None],
         (W2 @ np.asarray(att_dst2, np.float32))[:, None]], axis=1).astype(bfnp)
    b1a = np.asarray(b1, np.float32)[None, :].astype(bfnp)
    b2a = np.asarray(b2, np.float32)[None, :].astype(bfnp)

    inA = [{"xt": np.ascontiguousarray(xT[:, c * NSH:(c + 1) * NSH]),
            "w1e": w1ext} for c in range(CORES)]
    rA = run_bass_kernel_spmd(ncA, inA, core_ids=list(range(CORES)),
                              trace=_trace)
    h1full = np.empty((N + 1, E1), dtype=bfnp)
    for c in range(CORES):
        h1full[c * NSH:(c + 1) * NSH] = np.asarray(
            rA.results[c]["h1o"]).reshape(NSH, E1)
    sent = np.zeros((E1,), dtype=bfnp)
    sent.view(np.float32)[GS1] = -1e30
    sent.view(np.float32)[GD1] = -1e30
    h1full[N] = sent

    inB = [{"h1t": h1full, "w2e": w2ext, "b1d": b1a, "ix1": idx1[c]}
           for c in range(CORES)]
    rB = run_bass_kernel_spmd(ncB, inB, core_ids=list(range(CORES)),
                              trace=_trace)
    # interleave per-core position-slot outputs into global position order
    h2all = np.stack([np.asarray(rB.results[c]["h2o"]).reshape(NPC, E2)
                      for c in range(CORES)], axis=0)       # [C, NBLK*P, E2]
    h2full = np.empty((GSLOTS + 1, E2), dtype=bfnp)
    h2full[:GSLOTS] = (h2all.reshape(CORES, NBLK, P, E2)
                       .transpose(1, 0, 2, 3).reshape(GSLOTS, E2))
    sent2 = np.zeros((E2,), dtype=bfnp)
    sent2.view(np.float32)[GS2] = -1e30
    sent2.view(np.float32)[GD2] = -1e30
    h2full[GSLOTS] = sent2

    inC = [{"h2t": h2full, "b2d": b2a, "ix2": idx2[c]} for c in range(CORES)]
    rC = run_bass_kernel_spmd(ncC, inC, core_ids=list(range(CORES)),
                              trace=_trace)
    LAST_RESULT = (rA, rB, rC)

    out = np.zeros((N, OUT_F), dtype=np.float32)
    pp = np.arange(P)
    for c in range(CORES):
        oc = np.asarray(rC.results[c]["outp"]).reshape(NPC, OUT_F)
        for j in range(NBLK):
            g0 = (j * CORES + c) * P
            gg = g0 + pp
            valid = gg < N
            out[order[gg[valid]]] = oc[j * P:(j + 1) * P][valid]
    return out
